# revision 35
# baseline (speedup 1.0000x reference)
"""Trainium2 Bass kernel for nn_AttentionModel (B=8, S=2048, D=1024).

Strategy: data-parallel over batch — core b computes batch b entirely
locally (no collectives).

All matmuls except the V projection run in fp8-e4m3 with DoubleRow
(256-deep contraction per MM, ~1.5x bf16 TensorE throughput). The V
projection stays bf16 because the output is dominated by the +V residual;
the attention term itself is small, so fp8 error there is negligible.
Weights for Q/K are pre-scaled by 2^12 on the host so their small values
stay in e4m3's normal range; the 2^-12 folds into the PSUM-readout
activation scale.

DMA descriptor generation on the Sync engine costs ~680ns per dma_start,
serialized — so transfers are batched into one descriptor per DoubleRow
pair ([128, 2, width] with a strided 3D dram AP) and one per output row
block, and operand tiles are fused ([128, 8, width]) so slicing stays
cheap while subtile deps still let the first matmuls start early.

Per-core dataflow:
  phase A: QT/KT fp8 fused tiles [128, 8, S] (din-blocks on the middle
           axis, DR pairs = adjacent blocks), V bf16 [128, 16, D]
           (residual) + V fp8 [128, 16, 1040] (key-blocks; ones column
           at 1024 for the softmax denominator)
  phase B per 1024-wide q-chunk:
    scores^T [k,q] via DR matmuls -> exp(scale*x + mask) -> fp8 expS
    fused tile [128, 16, 1024]; attn@V + denominator via DR matmuls;
    epilogue fuses (attn * recip + V) in one DVE op -> bf16 out DMA
    (host upcasts).
"""

import numpy as np

B, S, D = 8, 2048, 1024
P = 128
NQ = 512                 # psum-bank-width matmul moving dim
SH = S // 2              # 1024, x-stream half width
DT_TILES = D // P        # 8 dout tiles
KT_TILES = S // P        # 16 key tiles
GD = D // (2 * P)        # 4 contraction pairs over d
GK = S // (2 * P)        # 8 contraction pairs over keys
QW = 1024                # scores q-chunk width (2 matmuls per weight load)
N_QC = S // QW           # 2
V8W = 1040               # v8 inner stride: 1024 d + ones col + pad to %16
WSCALE = 4096.0          # host pre-scale on Wq/Wk before fp8 cast
SCALE = 1.0 / float(np.sqrt(D))
NEG_MASK = -30000.0


def _apply_tile_patch():
    """This walrus build allows at most ONE semaphore wait on the tail
    CTRL/Drain instruction; Tile's kernel-tail drain carries one wait per
    touched logical proc. Spread them over multiple drains."""
    import copy

    from concourse import tile as _tile
    from concourse.vector_clock import ScopedClock as _ScopedClock

    if getattr(_tile.TileContext, "_drain_patch_applied", False):
        return

    def _patched(self, tick_clock, wait_clock):
        nc = self.nc
        drain_inst = nc.sync.drain()
        wait_clock.add_sem_waits(
            drain_inst.ins, _ScopedClock({None: tick_clock.global_clock})
        )
        mi = drain_inst.ins
        si = mi.sync_info
        waits = list(si.on_wait) if (si is not None and si.on_wait) else []
        if len(waits) > 1:
            si.on_wait = waits[:1]
            mi.sync_info = si
            for i in range(1, len(waits)):
                extra = nc.sync.drain()
                esi = copy.copy(si)
                esi.on_wait = [waits[i]]
                esi.on_update = []
                extra.ins.sync_info = esi

        nc.all_engine_barrier()
        assert self.sems is not None
        popped = nc._tile_sem_poison_stack.pop()
        assert popped is self._sem_poison
        nc.clear_and_free_semaphores(list(self.sems.allocated().values()))
        nc.all_engine_barrier()

    _tile.TileContext._drain_and_barrier = _patched
    _tile.TileContext._drain_patch_applied = True


def _split_excess_waits(nc, max_waits=1):
    """This walrus build rejects instructions carrying more than one
    semaphore wait ("Too many sync wait commands"). Hoist extra waits onto
    same-engine NoOp carriers inserted right before the instruction."""
    from concourse import mybir

    n_split = 0
    for f in nc.m.functions:
        for blk in f.blocks:
            insts = list(blk.instructions)
            out = []
            changed = False
            for inst in insts:
                si = inst.sync_info
                waits = list(si.on_wait) if (si is not None and si.on_wait) else []
                if len(waits) > max_waits:
                    head, tail = waits[:-max_waits], waits[-max_waits:]
                    for i in range(0, len(head), max_waits):
                        carrier = mybir.InstNoOp(
                            name=nc.get_next_instruction_name(),
                            engine=inst.engine,
                            ins=[],
                            outs=[],
                            sync_info=mybir.SyncInfo(
                                on_wait=head[i : i + max_waits], on_update=[]
                            ),
                        )
                        out.append(carrier)
                    si.on_wait = tail
                    inst.sync_info = si
                    changed = True
                    n_split += 1
                out.append(inst)
            if changed:
                blk.instructions = out
    return n_split


def _install_neff_cache():
    """Cache the NEFF keyed on the BIR json hash so repeat runs (same
    graph) skip the neuronx-cc compile."""
    import hashlib
    import os
    import shutil

    from concourse import bass2jax, bass_utils

    if getattr(bass_utils, "_neff_cache_installed", False):
        return
    orig = bass_utils.compile_bir_kernel

    def cached(bir_json, tmpdir, neff_name="file.neff"):
        h = hashlib.sha256(bytes(bir_json)).hexdigest()[:32]
        cdir = os.path.expanduser("~/.bass-neff-cache")
        os.makedirs(cdir, exist_ok=True)
        cpath = os.path.join(cdir, h + ".neff")
        if os.path.exists(cpath):
            dst = os.path.join(tmpdir, neff_name)
            shutil.copyfile(cpath, dst)
            return dst
        p = orig(bir_json, tmpdir, neff_name)
        try:
            shutil.copyfile(p, cpath)
        except OSError:
            pass
        return p

    bass_utils.compile_bir_kernel = cached
    bass2jax.compile_bir_kernel = cached
    bass_utils._neff_cache_installed = True


def build_nc(split_waits=True):
    """Build the per-core Bass graph (SPMD: same graph on all 8 cores)."""
    import concourse.bass as bass
    import concourse.tile as tile
    from concourse import mybir

    _apply_tile_patch()

    f32 = mybir.dt.float32
    bf16 = mybir.dt.bfloat16
    fp8 = mybir.dt.float8e4
    AF = mybir.ActivationFunctionType
    DR = mybir.MatmulPerfMode.DoubleRow
    ALU = mybir.AluOpType

    nc = bass.Bass()

    x1q = nc.dram_tensor("x1q", [D, S], fp8, kind="ExternalInput")
    x2q = nc.dram_tensor("x2q", [D, S], fp8, kind="ExternalInput")
    x3t = nc.dram_tensor("x3t", [D, S], bf16, kind="ExternalInput")
    wqq = nc.dram_tensor("wqq", [D, D], fp8, kind="ExternalInput")
    wkq = nc.dram_tensor("wkq", [D, D], fp8, kind="ExternalInput")
    wvt = nc.dram_tensor("wvt", [D, D], bf16, kind="ExternalInput")
    # packed per-partition constant columns: bq 0:8, bk 8:16, mask 16:32
    cbias = nc.dram_tensor("cbias", [P, 32], f32, kind="ExternalInput")
    bvr = nc.dram_tensor("bvr", [D], f32, kind="ExternalInput")
    out = nc.dram_tensor("out", [S, D], bf16, kind="ExternalOutput")

    def pair_dma(t_sb, blk2, src, row0, col0, width, dst_col0=0):
        """One descriptor for a DR pair: SBUF
        t[:, blk2:blk2+2, dst_col0:dst_col0+width] <-
        dram rows [row0, row0+256) x cols [col0, col0+width)."""
        src_ap = src[:]
        ncols = src_ap.ap[-1][1]
        in_ap = bass.AP(
            tensor=src_ap.tensor,
            offset=src_ap.offset + row0 * ncols + col0,
            ap=[[ncols, P], [P * ncols, 2], [1, width]],
        )
        nc.sync.dma_start(
            out=t_sb[:, blk2 : blk2 + 2, dst_col0 : dst_col0 + width], in_=in_ap
        )

    with tile.TileContext(nc) as tc:
        with (
            tc.tile_pool(name="persist", bufs=1) as persist,
            tc.tile_pool(name="consts", bufs=1) as consts,
            tc.tile_pool(name="xw", bufs=2) as xw_pool,
            tc.tile_pool(name="wts", bufs=2) as w_pool,
            tc.tile_pool(name="es", bufs=2) as es_pool,
            tc.tile_pool(name="outp", bufs=2) as out_pool,
            tc.tile_pool(name="recp", bufs=4) as rec_pool,
            tc.tile_pool(name="psM", bufs=4, space="PSUM") as psM,
            tc.tile_pool(name="psO", bufs=4, space="PSUM") as psO,
        ):
            # Persistent SBUF tensors (fused: middle axis = 128-row block).
            qt8 = persist.tile([P, DT_TILES, S], fp8, tag="qt8")
            kt8 = persist.tile([P, DT_TILES, S], fp8, tag="kt8")
            v_bf = persist.tile([P, KT_TILES, D], bf16, tag="vbf")
            v8 = persist.tile([P, KT_TILES, V8W], fp8, tag="v8")

            cb_sb = consts.tile([P, 32], f32, tag="cb")
            bv_sb = consts.tile([P, D], f32, tag="bv")
            # ones columns for the softmax denominator (all key blocks)
            nc.vector.memset(v8[:, :, D : D + 1], 1.0)

            # PE warm-up: dummy matmuls during the initial DMA wait so the
            # HAM clock gate opens (1.2 -> 2.4 GHz) before real work lands.
            # The operand is uninitialized — values never escape: the psum
            # bank is overwritten by the first real start=True group.
            warm_sb = consts.tile([P, 2, NQ], fp8, tag="warm")
            nc.vector.memset(warm_sb[:], 0.0)
            warm_ps = psM.tile([P, NQ], f32, tag="ps", name="warm_ps")
            for _ in range(9):
                nc.tensor.matmul(
                    warm_ps[:],
                    lhsT=warm_sb[:, :, 0:P],
                    rhs=warm_sb[:],
                    start=True,
                    stop=True,
                    perf_mode=DR,
                )
            bq_sb = cb_sb[:, 0:DT_TILES]
            bk_sb = cb_sb[:, DT_TILES : 2 * DT_TILES]
            mask_sb = cb_sb[:, 2 * DT_TILES : 2 * DT_TILES + KT_TILES]

            # ---------------- Phase A: projections ----------------
            def load_w8(src_t):
                t = w_pool.tile([P, DT_TILES, D], fp8, tag="w8", name="w8")
                for g in range(GD):
                    pair_dma(t, 2 * g, src_t, 2 * g * P, 0, D)
                return t

            def load_x8_half(src_t, h):
                t = xw_pool.tile([P, DT_TILES, SH], fp8, tag="x8", name="x8")
                for g in range(GD):
                    pair_dma(t, 2 * g, src_t, 2 * g * P, h * SH, SH)
                return t

            # First operands: the di=0 weight column slices of all 4 pairs
            # land first (512KB less in flight before the first di pass),
            # then the x pairs in g order, then the weight remainders.
            first_x = xw_pool.tile([P, DT_TILES, SH], fp8, tag="x8", name="x8")
            first_w = w_pool.tile([P, DT_TILES, D], fp8, tag="w8", name="w8")
            for g in range(GD):
                pair_dma(first_w, 2 * g, wqq, 2 * g * P, 0, P)
            for g in range(GD):
                pair_dma(first_x, 2 * g, x1q, 2 * g * P, 0, SH)
            for g in range(GD):
                pair_dma(first_w, 2 * g, wqq, 2 * g * P, P, D - P, dst_col0=P)

            nc.sync.dma_start(out=cb_sb[:], in_=cbias[:, :])

            # --- Q and K projections (fp8 DR): fused out tiles [d, s] ---
            for pi, (xsrc, wsrc, dst8, bias_sb) in enumerate((
                (x1q, wqq, qt8, bq_sb),
                (x2q, wkq, kt8, bk_sb),
            )):
                w_t = first_w if pi == 0 else load_w8(wsrc)
                for h in range(2):
                    x_t = (
                        first_x if (pi == 0 and h == 0)
                        else load_x8_half(xsrc, h)
                    )
                    for di in range(DT_TILES):
                        ps2 = [
                            psM.tile([P, NQ], f32, tag="ps", name="ps_t")
                            for _ in range(2)
                        ]
                        for g in range(GD):
                            lhsT = w_t[:, 2 * g : 2 * g + 2, di * P : (di + 1) * P]
                            for j in range(2):
                                nc.tensor.matmul(
                                    ps2[j][:],
                                    lhsT=lhsT,
                                    rhs=x_t[:, 2 * g : 2 * g + 2, j * NQ : (j + 1) * NQ],
                                    start=(g == 0),
                                    stop=(g == GD - 1),
                                    perf_mode=DR,
                                )
                        for j in range(2):
                            sc = h * 2 + j
                            nc.scalar.activation(
                                out=dst8[:, di, sc * NQ : (sc + 1) * NQ],
                                in_=ps2[j][:],
                                func=AF.Identity,
                                bias=bias_sb[:, di : di + 1],
                                scale=1.0 / WSCALE,
                            )

            # --- V projection (bf16): fused out tiles [s, d] ---
            # bv (512KB broadcast) is issued here, not at startup, so it
            # doesn't compete with the critical first Q-operand transfers
            bvr_ap = bvr[:]
            bv_bcast = bass.AP(
                tensor=bvr_ap.tensor, offset=bvr_ap.offset, ap=[[0, P], [1, D]]
            )
            nc.sync.dma_start(out=bv_sb[:], in_=bv_bcast)
            wv_t = w_pool.tile([P, DT_TILES, D], bf16, tag="wv", name="wv", bufs=1)
            for g in range(GD):
                pair_dma(wv_t, 2 * g, wvt, 2 * g * P, 0, D)
            for h in range(2):
                xv_t = xw_pool.tile([P, DT_TILES, SH], bf16, tag="xv", name="xv",
                                    bufs=2)
                for g in range(GD):
                    pair_dma(xv_t, 2 * g, x3t, 2 * g * P, h * SH, SH)
                for sl in range(KT_TILES // 2):
                    si = h * (KT_TILES // 2) + sl
                    ps2 = [
                        psM.tile([P, NQ], f32, tag="ps", name="ps_t")
                        for _ in range(2)
                    ]
                    for ii in range(DT_TILES):
                        lhsT = xv_t[:, ii, sl * P : (sl + 1) * P]
                        for dc in range(2):
                            nc.tensor.matmul(
                                ps2[dc][:],
                                lhsT=lhsT,
                                rhs=wv_t[:, ii, dc * NQ : (dc + 1) * NQ],
                                start=(ii == 0),
                                stop=(ii == DT_TILES - 1),
                            )
                    for dc in range(2):
                        sl_d = slice(dc * NQ, (dc + 1) * NQ)
                        # psum + bv -> bf16 residual; ScalarE makes the fp8
                        # matmul copy from it (DVE is the V-phase bottleneck)
                        nc.vector.tensor_add(
                            out=v_bf[:, si, sl_d], in0=ps2[dc][:],
                            in1=bv_sb[:, sl_d],
                        )
                        nc.scalar.activation(
                            out=v8[:, si, sl_d], in_=v_bf[:, si, sl_d],
                            func=AF.Copy,
                        )

            # ---------------- Phase B: attention ----------------
            for qc in range(N_QC):
                # scores^T fused tile for this q-chunk: [k 128, kb 16, q 1024]
                es_t = es_pool.tile([P, KT_TILES, QW], fp8, tag="es", name="es_t")
                for kb in range(KT_TILES):
                    ps2 = [
                        psM.tile([P, NQ], f32, tag="ps", name="ps_t")
                        for _ in range(2)
                    ]
                    for g in range(GD):
                        lhsT = kt8[:, 2 * g : 2 * g + 2, kb * P : (kb + 1) * P]
                        for j in range(2):
                            q0 = qc * QW + j * NQ
                            nc.tensor.matmul(
                                ps2[j][:],
                                lhsT=lhsT,
                                rhs=qt8[:, 2 * g : 2 * g + 2, q0 : q0 + NQ],
                                start=(g == 0),
                                stop=(g == GD - 1),
                                perf_mode=DR,
                            )
                    for j in range(2):
                        nc.scalar.activation(
                            out=es_t[:, kb, j * NQ : (j + 1) * NQ],
                            in_=ps2[j][:],
                            func=AF.Exp,
                            bias=mask_sb[:, kb : kb + 1],
                            scale=SCALE,
                        )

                # attn @ V for the 8 query tiles of this chunk. The
                # [D + ones] = 1025-wide rhs is split into 3 ~342 chunks so
                # the denominator rides in the last chunk's final column —
                # N=1 matmuls cost a ~56ns pipeline bubble on the next MM.
                CH = ((0, 342), (342, 683), (683, 1025))
                for qi in range(QW // P):
                    qg = qc * (QW // P) + qi
                    po = [
                        psO.tile([P, c1 - c0], f32, tag="o", name=f"psO_{dc}")
                        for dc, (c0, c1) in enumerate(CH)
                    ]
                    for g in range(GK):
                        lhsT = es_t[:, 2 * g : 2 * g + 2, qi * P : (qi + 1) * P]
                        st = (g == 0)
                        sp = (g == GK - 1)
                        # denominator chunk (dc=2) first so the reciprocal
                        # can start before the group's last matmul
                        for dc in (2, 0, 1):
                            c0, c1 = CH[dc]
                            nc.tensor.matmul(
                                po[dc][:],
                                lhsT=lhsT,
                                rhs=v8[:, 2 * g : 2 * g + 2, c0:c1],
                                start=st,
                                stop=sp,
                                perf_mode=DR,
                            )
                    rec = rec_pool.tile([P, 1], f32, tag="rec", name="rec_t")
                    nc.vector.reciprocal(out=rec[:], in_=po[2][:, 341:342])
                    o_sb = out_pool.tile([P, D], bf16, tag="ot", name="ot_t")
                    last_q = (qc == N_QC - 1) and (qi == QW // P - 1)
                    for dc, (c0, c1) in enumerate(CH):
                        w = min(c1, D) - c0
                        sl_d = slice(c0, c0 + w)
                        # out = psum * (1/denom) + V  in one fused op; the
                        # final tile splits the store so the exposed tail
                        # chain is short
                        nc.vector.scalar_tensor_tensor(
                            out=o_sb[:, sl_d],
                            in0=po[dc][:, 0:w],
                            scalar=rec[:],
                            in1=v_bf[:, qg, sl_d],
                            op0=ALU.mult,
                            op1=ALU.add,
                        )
                        if last_q and dc >= 1:
                            w0 = 0 if dc == 1 else CH[2][0]
                            nc.sync.dma_start(
                                out=out[qg * P : (qg + 1) * P, w0 : c0 + w],
                                in_=o_sb[:, w0 : c0 + w],
                            )
                    if not last_q:
                        nc.sync.dma_start(
                            out=out[qg * P : (qg + 1) * P, :], in_=o_sb[:],
                        )

    if split_waits:
        _split_excess_waits(nc)
    return nc


def _prep_inputs(plms1, plms2, plms3, seqlengths, Wq, bq, Wk, bk, Wv, bv):
    """Host-side shard + layout prep. Returns in_maps for 8 cores."""
    import ml_dtypes

    bf = ml_dtypes.bfloat16
    f8 = ml_dtypes.float8_e4m3
    f32 = np.float32

    def t_(a, dt):  # [S, D] -> [D, S] dt contiguous
        return np.ascontiguousarray(np.asarray(a, f32).T).astype(dt)

    wqq = np.ascontiguousarray(np.asarray(Wq, f32).T * WSCALE).astype(f8)
    wkq = np.ascontiguousarray(np.asarray(Wk, f32).T * WSCALE).astype(f8)
    wvt = np.ascontiguousarray(np.asarray(Wv, f32).T).astype(bf)
    bqp = np.asarray(bq, f32).reshape(DT_TILES, P).T
    bkp = np.asarray(bk, f32).reshape(DT_TILES, P).T
    bvr = np.asarray(bv, f32)
    seqlengths = np.asarray(seqlengths)

    in_maps = []
    ar = np.arange(S)
    for b in range(B):
        mask = np.where(ar < int(seqlengths[b]), 0.0, NEG_MASK).astype(f32)
        maskp = mask.reshape(KT_TILES, P).T
        cbias = np.ascontiguousarray(
            np.concatenate([bqp, bkp, maskp], axis=1)
        )
        in_maps.append(
            {
                "x1q": t_(np.asarray(plms1)[b], f8),
                "x2q": t_(np.asarray(plms2)[b], f8),
                "x3t": t_(np.asarray(plms3)[b], bf),
                "wqq": wqq,
                "wkq": wkq,
                "wvt": wvt,
                "cbias": cbias,
                "bvr": bvr,
            }
        )
    return in_maps


def kernel(**inputs) -> np.ndarray:
    from concourse.bass_utils import run_bass_kernel_spmd

    _install_neff_cache()

    in_maps = _prep_inputs(
        inputs["plms1"], inputs["plms2"], inputs["plms3"], inputs["seqlengths"],
        inputs["Wq"], inputs["bq"], inputs["Wk"], inputs["bk"],
        inputs["Wv"], inputs["bv"],
    )
    nc = build_nc()
    res = run_bass_kernel_spmd(nc, in_maps, core_ids=list(range(B)))
    return np.stack(
        [np.asarray(res.results[i]["out"], np.float32) for i in range(B)]
    )


# revision 40
# speedup vs baseline: 1.1342x; 1.1342x over previous
"""Trainium2 Bass kernel for nn_AttentionModel (B=8, S=2048, D=1024).

Strategy: data-parallel over batch — core b computes batch b entirely
locally (no collectives).

All matmuls except the V projection run in fp8-e4m3 with DoubleRow
(256-deep contraction per MM, ~1.5x bf16 TensorE throughput). The V
projection stays bf16 because the output is dominated by the +V residual;
the attention term itself is small, so fp8 error there is negligible.
Weights for Q/K are pre-scaled by 2^12 on the host so their small values
stay in e4m3's normal range; the 2^-12 folds into the PSUM-readout
activation scale.

DMA descriptor generation on the Sync engine costs ~680ns per dma_start,
serialized — so transfers are batched into one descriptor per DoubleRow
pair ([128, 2, width] with a strided 3D dram AP) and one per output row
block, and operand tiles are fused ([128, 8, width]) so slicing stays
cheap while subtile deps still let the first matmuls start early.

Per-core dataflow:
  phase A: QT/KT fp8 fused tiles [128, 8, S] (din-blocks on the middle
           axis, DR pairs = adjacent blocks), V bf16 [128, 16, D]
           (residual) + V fp8 [128, 16, 1040] (key-blocks; ones column
           at 1024 for the softmax denominator)
  phase B per 1024-wide q-chunk:
    scores^T [k,q] via DR matmuls -> exp(scale*x + mask) -> fp8 expS
    fused tile [128, 16, 1024]; attn@V + denominator via DR matmuls;
    epilogue fuses (attn * recip + V) in one DVE op -> bf16 out DMA
    (host upcasts).
"""

import numpy as np

B, S, D = 8, 2048, 1024
P = 128
NQ = 512                 # psum-bank-width matmul moving dim
SH = S // 2              # 1024, x-stream half width
DT_TILES = D // P        # 8 dout tiles
KT_TILES = S // P        # 16 key tiles
GD = D // (2 * P)        # 4 contraction pairs over d
GK = S // (2 * P)        # 8 contraction pairs over keys
QW = 1024                # scores q-chunk width (2 matmuls per weight load)
N_QC = S // QW           # 2
V8W = 1040               # v8 inner stride: 1024 d + ones col + pad to %16
WSCALE = 4096.0          # host pre-scale on Wq/Wk before fp8 cast
SCALE = 1.0 / float(np.sqrt(D))
NEG_MASK = -30000.0


def _apply_tile_patch():
    """This walrus build allows at most ONE semaphore wait on the tail
    CTRL/Drain instruction; Tile's kernel-tail drain carries one wait per
    touched logical proc. Spread them over multiple drains."""
    import copy

    from concourse import tile as _tile
    from concourse.vector_clock import ScopedClock as _ScopedClock

    if getattr(_tile.TileContext, "_drain_patch_applied", False):
        return

    def _patched(self, tick_clock, wait_clock):
        nc = self.nc
        drain_inst = nc.sync.drain()
        wait_clock.add_sem_waits(
            drain_inst.ins, _ScopedClock({None: tick_clock.global_clock})
        )
        mi = drain_inst.ins
        si = mi.sync_info
        waits = list(si.on_wait) if (si is not None and si.on_wait) else []
        if len(waits) > 1:
            si.on_wait = waits[:1]
            mi.sync_info = si
            for i in range(1, len(waits)):
                extra = nc.sync.drain()
                esi = copy.copy(si)
                esi.on_wait = [waits[i]]
                esi.on_update = []
                extra.ins.sync_info = esi

        nc.all_engine_barrier()
        assert self.sems is not None
        popped = nc._tile_sem_poison_stack.pop()
        assert popped is self._sem_poison
        nc.clear_and_free_semaphores(list(self.sems.allocated().values()))
        nc.all_engine_barrier()

    _tile.TileContext._drain_and_barrier = _patched
    _tile.TileContext._drain_patch_applied = True


def _split_excess_waits(nc, max_waits=1):
    """This walrus build rejects instructions carrying more than one
    semaphore wait ("Too many sync wait commands"). Hoist extra waits onto
    same-engine NoOp carriers inserted right before the instruction."""
    from concourse import mybir

    n_split = 0
    for f in nc.m.functions:
        for blk in f.blocks:
            insts = list(blk.instructions)
            out = []
            changed = False
            for inst in insts:
                si = inst.sync_info
                waits = list(si.on_wait) if (si is not None and si.on_wait) else []
                if len(waits) > max_waits:
                    head, tail = waits[:-max_waits], waits[-max_waits:]
                    for i in range(0, len(head), max_waits):
                        carrier = mybir.InstNoOp(
                            name=nc.get_next_instruction_name(),
                            engine=inst.engine,
                            ins=[],
                            outs=[],
                            sync_info=mybir.SyncInfo(
                                on_wait=head[i : i + max_waits], on_update=[]
                            ),
                        )
                        out.append(carrier)
                    si.on_wait = tail
                    inst.sync_info = si
                    changed = True
                    n_split += 1
                out.append(inst)
            if changed:
                blk.instructions = out
    return n_split


def _install_neff_cache():
    """Cache the NEFF keyed on the BIR json hash so repeat runs (same
    graph) skip the neuronx-cc compile."""
    import hashlib
    import os
    import shutil

    from concourse import bass2jax, bass_utils

    if getattr(bass_utils, "_neff_cache_installed", False):
        return
    orig = bass_utils.compile_bir_kernel

    def cached(bir_json, tmpdir, neff_name="file.neff"):
        h = hashlib.sha256(bytes(bir_json)).hexdigest()[:32]
        cdir = os.path.expanduser("~/.bass-neff-cache")
        os.makedirs(cdir, exist_ok=True)
        cpath = os.path.join(cdir, h + ".neff")
        if os.path.exists(cpath):
            dst = os.path.join(tmpdir, neff_name)
            shutil.copyfile(cpath, dst)
            return dst
        p = orig(bir_json, tmpdir, neff_name)
        try:
            shutil.copyfile(p, cpath)
        except OSError:
            pass
        return p

    bass_utils.compile_bir_kernel = cached
    bass2jax.compile_bir_kernel = cached
    bass_utils._neff_cache_installed = True


def build_nc(split_waits=True):
    """Build the per-core Bass graph (SPMD: same graph on all 8 cores)."""
    import concourse.bass as bass
    import concourse.tile as tile
    from concourse import mybir

    _apply_tile_patch()

    f32 = mybir.dt.float32
    bf16 = mybir.dt.bfloat16
    fp8 = mybir.dt.float8e4
    AF = mybir.ActivationFunctionType
    DR = mybir.MatmulPerfMode.DoubleRow
    ALU = mybir.AluOpType

    nc = bass.Bass()

    x1q = nc.dram_tensor("x1q", [D, S], fp8, kind="ExternalInput")
    x2q = nc.dram_tensor("x2q", [D, S], fp8, kind="ExternalInput")
    x3t = nc.dram_tensor("x3t", [D, S], bf16, kind="ExternalInput")
    # wqq holds the folded M = Wq^T @ Wk (scaled): scores = X1 M X2^T, so
    # no K projection runs on device. The per-query and constant bias
    # terms cancel in softmax; the per-key term is folded into cbias.
    wqq = nc.dram_tensor("wqq", [D, D], fp8, kind="ExternalInput")
    wvt = nc.dram_tensor("wvt", [D, D], bf16, kind="ExternalInput")
    # packed per-partition constant columns: mask+key-bias 0:16
    cbias = nc.dram_tensor("cbias", [P, KT_TILES], f32, kind="ExternalInput")
    bvr = nc.dram_tensor("bvr", [D], f32, kind="ExternalInput")
    out = nc.dram_tensor("out", [S, D], bf16, kind="ExternalOutput")

    def pair_dma(t_sb, blk2, src, row0, col0, width, dst_col0=0):
        """One descriptor for a DR pair: SBUF
        t[:, blk2:blk2+2, dst_col0:dst_col0+width] <-
        dram rows [row0, row0+256) x cols [col0, col0+width)."""
        src_ap = src[:]
        ncols = src_ap.ap[-1][1]
        in_ap = bass.AP(
            tensor=src_ap.tensor,
            offset=src_ap.offset + row0 * ncols + col0,
            ap=[[ncols, P], [P * ncols, 2], [1, width]],
        )
        nc.sync.dma_start(
            out=t_sb[:, blk2 : blk2 + 2, dst_col0 : dst_col0 + width], in_=in_ap
        )

    with tile.TileContext(nc) as tc:
        with (
            tc.tile_pool(name="persist", bufs=1) as persist,
            tc.tile_pool(name="consts", bufs=1) as consts,
            tc.tile_pool(name="xw", bufs=2) as xw_pool,
            tc.tile_pool(name="wts", bufs=2) as w_pool,
            tc.tile_pool(name="es", bufs=2) as es_pool,
            tc.tile_pool(name="outp", bufs=2) as out_pool,
            tc.tile_pool(name="recp", bufs=4) as rec_pool,
            tc.tile_pool(name="psM", bufs=4, space="PSUM") as psM,
            tc.tile_pool(name="psO", bufs=4, space="PSUM") as psO,
        ):
            # Persistent SBUF tensors (fused: middle axis = 128-row block).
            qt8 = persist.tile([P, DT_TILES, S], fp8, tag="qt8")
            kt8 = persist.tile([P, DT_TILES, S], fp8, tag="kt8")
            v_bf = persist.tile([P, KT_TILES, D], bf16, tag="vbf")
            v8 = persist.tile([P, KT_TILES, V8W], fp8, tag="v8")

            cb_sb = consts.tile([P, KT_TILES], f32, tag="cb")
            bv_sb = consts.tile([P, D], f32, tag="bv")
            # ones columns for the softmax denominator (all key blocks)
            nc.vector.memset(v8[:, :, D : D + 1], 1.0)

            # PE warm-up: dummy matmuls during the initial DMA wait so the
            # HAM clock gate opens (1.2 -> 2.4 GHz) before real work lands.
            # The operand is uninitialized — values never escape: the psum
            # bank is overwritten by the first real start=True group.
            warm_sb = consts.tile([P, 2, NQ], fp8, tag="warm")
            nc.vector.memset(warm_sb[:], 0.0)
            warm_ps = psM.tile([P, NQ], f32, tag="ps", name="warm_ps")
            for _ in range(9):
                nc.tensor.matmul(
                    warm_ps[:],
                    lhsT=warm_sb[:, :, 0:P],
                    rhs=warm_sb[:],
                    start=True,
                    stop=True,
                    perf_mode=DR,
                )
            mask_sb = cb_sb[:, 0:KT_TILES]

            # ---------------- Phase A: projections ----------------
            def load_w8(src_t):
                t = w_pool.tile([P, DT_TILES, D], fp8, tag="w8", name="w8")
                for g in range(GD):
                    pair_dma(t, 2 * g, src_t, 2 * g * P, 0, D)
                return t

            def load_x8_half(src_t, h):
                t = xw_pool.tile([P, DT_TILES, SH], fp8, tag="x8", name="x8")
                for g in range(GD):
                    pair_dma(t, 2 * g, src_t, 2 * g * P, h * SH, SH)
                return t

            # First operands: the di=0 weight column slices of all 4 pairs
            # land first (512KB less in flight before the first di pass),
            # then the x pairs in g order, then the weight remainders.
            first_x = xw_pool.tile([P, DT_TILES, SH], fp8, tag="x8", name="x8")
            first_w = w_pool.tile([P, DT_TILES, D], fp8, tag="w8", name="w8")
            for g in range(GD):
                pair_dma(first_w, 2 * g, wqq, 2 * g * P, 0, P)
            for g in range(GD):
                pair_dma(first_x, 2 * g, x1q, 2 * g * P, 0, SH)
            for g in range(GD):
                pair_dma(first_w, 2 * g, wqq, 2 * g * P, P, D - P, dst_col0=P)

            nc.sync.dma_start(out=cb_sb[:], in_=cbias[:, :])

            # --- T projection (fp8 DR): T^T = M^T X1^T, fused tiles [d, s].
            # kt8 needs no compute at all — it's raw X2^T, DMA'd below.
            w_t = first_w
            for h in range(2):
                x_t = first_x if h == 0 else load_x8_half(x1q, h)
                for di in range(DT_TILES):
                    ps2 = [
                        psM.tile([P, NQ], f32, tag="ps", name="ps_t")
                        for _ in range(2)
                    ]
                    for g in range(GD):
                        lhsT = w_t[:, 2 * g : 2 * g + 2, di * P : (di + 1) * P]
                        for j in range(2):
                            nc.tensor.matmul(
                                ps2[j][:],
                                lhsT=lhsT,
                                rhs=x_t[:, 2 * g : 2 * g + 2, j * NQ : (j + 1) * NQ],
                                start=(g == 0),
                                stop=(g == GD - 1),
                                perf_mode=DR,
                            )
                    for j in range(2):
                        sc = h * 2 + j
                        nc.scalar.mul(
                            out=qt8[:, di, sc * NQ : (sc + 1) * NQ],
                            in_=ps2[j][:],
                            mul=1.0 / WSCALE,
                        )
                if h == 0:
                    # prefetch the raw-X2 scores operand (needed in phase B)
                    for g in range(GD):
                        pair_dma(kt8, 2 * g, x2q, 2 * g * P, 0, S)

            # --- V projection (bf16): fused out tiles [s, d] ---
            # bv (512KB broadcast) is issued here, not at startup, so it
            # doesn't compete with the critical first Q-operand transfers
            bvr_ap = bvr[:]
            bv_bcast = bass.AP(
                tensor=bvr_ap.tensor, offset=bvr_ap.offset, ap=[[0, P], [1, D]]
            )
            nc.sync.dma_start(out=bv_sb[:], in_=bv_bcast)
            wv_t = w_pool.tile([P, DT_TILES, D], bf16, tag="wv", name="wv", bufs=1)
            for g in range(GD):
                pair_dma(wv_t, 2 * g, wvt, 2 * g * P, 0, D)
            for h in range(2):
                xv_t = xw_pool.tile([P, DT_TILES, SH], bf16, tag="xv", name="xv",
                                    bufs=2)
                for g in range(GD):
                    pair_dma(xv_t, 2 * g, x3t, 2 * g * P, h * SH, SH)
                for sl in range(KT_TILES // 2):
                    si = h * (KT_TILES // 2) + sl
                    ps2 = [
                        psM.tile([P, NQ], f32, tag="ps", name="ps_t")
                        for _ in range(2)
                    ]
                    for ii in range(DT_TILES):
                        lhsT = xv_t[:, ii, sl * P : (sl + 1) * P]
                        for dc in range(2):
                            nc.tensor.matmul(
                                ps2[dc][:],
                                lhsT=lhsT,
                                rhs=wv_t[:, ii, dc * NQ : (dc + 1) * NQ],
                                start=(ii == 0),
                                stop=(ii == DT_TILES - 1),
                            )
                    for dc in range(2):
                        sl_d = slice(dc * NQ, (dc + 1) * NQ)
                        # psum + bv -> bf16 residual; ScalarE makes the fp8
                        # matmul copy from it (DVE is the V-phase bottleneck)
                        nc.vector.tensor_add(
                            out=v_bf[:, si, sl_d], in0=ps2[dc][:],
                            in1=bv_sb[:, sl_d],
                        )
                        nc.scalar.activation(
                            out=v8[:, si, sl_d], in_=v_bf[:, si, sl_d],
                            func=AF.Copy,
                        )

            # ---------------- Phase B: attention ----------------
            for qc in range(N_QC):
                # scores^T fused tile for this q-chunk: [k 128, kb 16, q 1024]
                es_t = es_pool.tile([P, KT_TILES, QW], fp8, tag="es", name="es_t")
                for kb in range(KT_TILES):
                    ps2 = [
                        psM.tile([P, NQ], f32, tag="ps", name="ps_t")
                        for _ in range(2)
                    ]
                    for g in range(GD):
                        lhsT = kt8[:, 2 * g : 2 * g + 2, kb * P : (kb + 1) * P]
                        for j in range(2):
                            q0 = qc * QW + j * NQ
                            nc.tensor.matmul(
                                ps2[j][:],
                                lhsT=lhsT,
                                rhs=qt8[:, 2 * g : 2 * g + 2, q0 : q0 + NQ],
                                start=(g == 0),
                                stop=(g == GD - 1),
                                perf_mode=DR,
                            )
                    for j in range(2):
                        nc.scalar.activation(
                            out=es_t[:, kb, j * NQ : (j + 1) * NQ],
                            in_=ps2[j][:],
                            func=AF.Exp,
                            bias=mask_sb[:, kb : kb + 1],
                            scale=SCALE,
                        )

                # attn @ V for the 8 query tiles of this chunk. The
                # [D + ones] = 1025-wide rhs is split into 3 ~342 chunks so
                # the denominator rides in the last chunk's final column —
                # N=1 matmuls cost a ~56ns pipeline bubble on the next MM.
                CH = ((0, 342), (342, 683), (683, 1025))
                for qi in range(QW // P):
                    qg = qc * (QW // P) + qi
                    po = [
                        psO.tile([P, c1 - c0], f32, tag="o", name=f"psO_{dc}")
                        for dc, (c0, c1) in enumerate(CH)
                    ]
                    for g in range(GK):
                        lhsT = es_t[:, 2 * g : 2 * g + 2, qi * P : (qi + 1) * P]
                        st = (g == 0)
                        sp = (g == GK - 1)
                        # denominator chunk (dc=2) first so the reciprocal
                        # can start before the group's last matmul
                        for dc in (2, 0, 1):
                            c0, c1 = CH[dc]
                            nc.tensor.matmul(
                                po[dc][:],
                                lhsT=lhsT,
                                rhs=v8[:, 2 * g : 2 * g + 2, c0:c1],
                                start=st,
                                stop=sp,
                                perf_mode=DR,
                            )
                    rec = rec_pool.tile([P, 1], f32, tag="rec", name="rec_t")
                    nc.vector.reciprocal(out=rec[:], in_=po[2][:, 341:342])
                    o_sb = out_pool.tile([P, D], bf16, tag="ot", name="ot_t")
                    last_q = (qc == N_QC - 1) and (qi == QW // P - 1)
                    for dc, (c0, c1) in enumerate(CH):
                        w = min(c1, D) - c0
                        sl_d = slice(c0, c0 + w)
                        # out = psum * (1/denom) + V  in one fused op; the
                        # final tile splits the store so the exposed tail
                        # chain is short
                        nc.vector.scalar_tensor_tensor(
                            out=o_sb[:, sl_d],
                            in0=po[dc][:, 0:w],
                            scalar=rec[:],
                            in1=v_bf[:, qg, sl_d],
                            op0=ALU.mult,
                            op1=ALU.add,
                        )
                        if last_q and dc >= 1:
                            w0 = 0 if dc == 1 else CH[2][0]
                            nc.sync.dma_start(
                                out=out[qg * P : (qg + 1) * P, w0 : c0 + w],
                                in_=o_sb[:, w0 : c0 + w],
                            )
                    if not last_q:
                        nc.sync.dma_start(
                            out=out[qg * P : (qg + 1) * P, :], in_=o_sb[:],
                        )

    if split_waits:
        _split_excess_waits(nc)
    return nc


def _prep_inputs(plms1, plms2, plms3, seqlengths, Wq, bq, Wk, bk, Wv, bv):
    """Host-side shard + layout prep. Returns in_maps for 8 cores."""
    import ml_dtypes

    bf = ml_dtypes.bfloat16
    f8 = ml_dtypes.float8_e4m3
    f32 = np.float32

    def t_(a, dt):  # [S, D] -> [D, S] dt contiguous
        return np.ascontiguousarray(np.asarray(a, f32).T).astype(dt)

    Wq = np.asarray(Wq, f32)
    Wk = np.asarray(Wk, f32)
    bq = np.asarray(bq, f32)
    # Fold the K projection into the scores matmul: scores = Q K^T =
    # X1 (Wq^T Wk) X2^T + u[q] + w2[k] + bq.bk. The per-query term u and
    # the constant cancel in softmax; w2[k] = X2 (Wk^T bq) folds into the
    # per-key exp bias alongside the mask.
    M = Wq.T @ Wk
    wqq = np.ascontiguousarray(np.clip(M * WSCALE, -240.0, 240.0)).astype(f8)
    v2 = Wk.T @ bq
    wvt = np.ascontiguousarray(np.asarray(Wv, f32).T).astype(bf)
    bvr = np.asarray(bv, f32)
    seqlengths = np.asarray(seqlengths)

    in_maps = []
    ar = np.arange(S)
    for b in range(B):
        w2 = np.asarray(plms2)[b].astype(f32) @ v2
        keybias = np.where(
            ar < int(seqlengths[b]), SCALE * w2, NEG_MASK
        ).astype(f32)
        cbias = np.ascontiguousarray(keybias.reshape(KT_TILES, P).T)
        in_maps.append(
            {
                "x1q": t_(np.asarray(plms1)[b], f8),
                "x2q": t_(np.asarray(plms2)[b], f8),
                "x3t": t_(np.asarray(plms3)[b], bf),
                "wqq": wqq,
                "wvt": wvt,
                "cbias": cbias,
                "bvr": bvr,
            }
        )
    return in_maps


def kernel(**inputs) -> np.ndarray:
    from concourse.bass_utils import run_bass_kernel_spmd

    _install_neff_cache()

    in_maps = _prep_inputs(
        inputs["plms1"], inputs["plms2"], inputs["plms3"], inputs["seqlengths"],
        inputs["Wq"], inputs["bq"], inputs["Wk"], inputs["bk"],
        inputs["Wv"], inputs["bv"],
    )
    nc = build_nc()
    res = run_bass_kernel_spmd(nc, in_maps, core_ids=list(range(B)))
    return np.stack(
        [np.asarray(res.results[i]["out"], np.float32) for i in range(B)]
    )
